# revision 10
# baseline (speedup 1.0000x reference)
"""VQ codebook (vector-quantization) kernel for one TRN2 chip (8 NeuronCores).

Data-parallel over the batch dim: each core handles 4 of the 32 batches
(4096 of the 32768 flattened latent vectors).  Per core:

  - distances d[n,k] = ||z_n||^2 + ||e_k||^2 - 2 z_n.e_k are computed so the
    fp32 rounding of the reference (fl(fl(A+B) - fl(2M))) is reproduced on the
    ulp(A)-grid:  the matmul accumulates 2M - round_g(B) in PSUM (grid-rounded
    rb rows are folded into the contraction as two extra rows, with a
    per-row binade select), then ACT adds the per-row -A bias in one rounding.
  - fp32 products are decomposed into three exact fp16 matmuls
    (zh*eh + zh*el + zl*eh) - fp16 matmul is bit-exact and runs 4x faster
    than native fp32 on the PE.
  - row argmin: DVE max8 + max_index (first-occurrence on ties, matching
    jnp.argmin).
  - z_q gathered transposed straight from an SBUF codebook table via the
    GPSIMD ap_gather custom op (d=2 packed c-pairs).
  - loss falls out of the row-min distances (sum of min d == SSE); the 8
    per-core partial sums are reduced on the host (the only "all-reduce").
"""

import sys

sys.path.insert(0, "/opt/trn_rl_repo")

import numpy as np

import concourse.bass as bass
import concourse.mybir as mybir
from concourse import bacc, tile
from concourse.bass_utils import run_bass_kernel_spmd
from concourse.library_config import ap_gather as _apg_lib

F32 = mybir.dt.float32
F16 = mybir.dt.float16
I16 = mybir.dt.int16
I32 = mybir.dt.int32
U16 = mybir.dt.uint16

N_CORES = 8
B, D, H, W = 32, 256, 32, 32
K = 1024
B_LOC = B // N_CORES          # 4 batches per core
N_LOC = B_LOC * H * W         # 4096 rows per core
NCHUNK = N_LOC // 128         # 32 chunks of 128 rows
G1, G2 = 2.0 ** -16, 2.0 ** -15


def _build_nc():
    nc = bacc.Bacc("TRN2", target_bir_lowering=False, debug=False)
    dram = {}

    def inp(name, shape, dt=F32):
        dram[name] = nc.dram_tensor(name, list(shape), dt, kind="ExternalInput")
        return dram[name]

    def outp(name, shape, dt=F32):
        dram[name] = nc.dram_tensor(name, list(shape), dt, kind="ExternalOutput")
        return dram[name]

    inp("zh2", [2, 128, N_LOC], F16)       # z high fp16, [c_half, c%128, n]
    inp("zl2", [2, 128, N_LOC], F16)       # z low  fp16
    inp("ehT", [2, 128, K], F16)           # (2e)^T high fp16, [c_half, c%128, k]
    inp("elT", [2, 128, K], F16)           # (2e)^T low  fp16
    inp("rbrow", [2, K], F16)              # [-rb1 ; -(rb2-rb1)]
    inp("onesrow", [1, N_LOC], F16)        # all-ones (partition 0 of sel2)
    inp("etab", [128, K, 2], F32)          # e packed c-pairs: [cp, k, ci]
    inp("ident16", [128, 128], F16)
    inp("ident32", [128, 128], F32)
    outp("zq", [B_LOC, D, H * W], F32)
    outp("idx", [NCHUNK, 128], I32)
    outp("lossv", [128, 1], F32)
    dram["drsel"] = nc.dram_tensor("drsel", [NCHUNK, 128], F16)  # bounce

    with tile.TileContext(nc) as tc:
        with (
            tc.tile_pool(name="big", bufs=1) as big,
            tc.tile_pool(name="nd_pool", bufs=3) as ndp,
            tc.tile_pool(name="scr", bufs=2) as scr,
            tc.tile_pool(name="psM", bufs=2, space=bass.MemorySpace.PSUM) as psMp,
            tc.tile_pool(name="psA", bufs=2, space=bass.MemorySpace.PSUM) as psAp,
            tc.tile_pool(name="psS", bufs=2, space=bass.MemorySpace.PSUM) as psSp,
        ):
            # ---- resident tensors ----
            zh_lo = big.tile([128, N_LOC], F16, name="zh_lo")
            zh_hi = big.tile([128, N_LOC], F16, name="zh_hi")
            zl_lo = big.tile([128, N_LOC], F16, name="zl_lo")
            zl_hi = big.tile([128, N_LOC], F16, name="zl_hi")
            eh_lo = big.tile([128, K], F16, name="eh_lo")
            eh_hi = big.tile([128, K], F16, name="eh_hi")
            el_lo = big.tile([128, K], F16, name="el_lo")
            el_hi = big.tile([128, K], F16, name="el_hi")
            rb_t = big.tile([2, K], F16, name="rb_t")
            id16 = big.tile([128, 128], F16, name="id16")
            id32 = big.tile([128, 128], F32, name="id32")
            etab = big.tile([128, K, 2], F32, name="etab")
            sel2 = big.tile([2, NCHUNK, 128], F16, name="sel2")
            A_all = big.tile([128, NCHUNK], F32, name="A_all")
            Alo = big.tile([128, NCHUNK], F32, name="Alo")
            Ahi = big.tile([128, NCHUNK], F32, name="Ahi")
            negA = big.tile([128, NCHUNK], F32, name="negA")
            max8a = big.tile([128, 8 * NCHUNK], F32, name="max8a")
            idx8a = big.tile([128, 8 * NCHUNK], U16, name="idx8a")
            wrp16 = big.tile([16, N_LOC // 16], U16, name="wrp16")
            wrp128 = big.tile([128, N_LOC // 16], U16, name="wrp128")
            zqp = big.tile([128, N_LOC, 2], F32, name="zqp")
            lossv = big.tile([128, 1], F32, name="lossv")
            idxf = big.tile([128, NCHUNK], F32, name="idxf")
            idxT = big.tile([NCHUNK, 128], I32, name="idxT")

            nc.sync.dma_start(zh_lo[:], dram["zh2"][0])
            nc.sync.dma_start(zh_hi[:], dram["zh2"][1])
            nc.sync.dma_start(zl_lo[:], dram["zl2"][0])
            nc.sync.dma_start(zl_hi[:], dram["zl2"][1])
            nc.sync.dma_start(eh_lo[:], dram["ehT"][0])
            nc.sync.dma_start(eh_hi[:], dram["ehT"][1])
            nc.sync.dma_start(el_lo[:], dram["elT"][0])
            nc.sync.dma_start(el_hi[:], dram["elT"][1])
            nc.sync.dma_start(rb_t[:], dram["rbrow"][:])
            nc.sync.dma_start(id16[:], dram["ident16"][:])
            nc.sync.dma_start(id32[:], dram["ident32"][:])
            nc.sync.dma_start(etab[:], dram["etab"][:])
            nc.sync.dma_start(sel2[0:1, :, :], dram["onesrow"][:].rearrange("p (j n) -> p j n", j=NCHUNK))

            # ---- A pass: A[n] = sum_c zh[c,n]^2 via PE transpose + ACT square-accum
            for j in range(NCHUNK):
                psA = psAp.tile([128, 256], F16, name="psA")
                nc.tensor.transpose(psA[:, 0:128], zh_lo[:, 128 * j:128 * (j + 1)], id16[:])
                nc.tensor.transpose(psA[:, 128:256], zh_hi[:, 128 * j:128 * (j + 1)], id16[:])
                sqs = scr.tile([128, 256], F16, name="sqs", tag="sqs")
                nc.scalar.activation(sqs[:], psA[:], mybir.ActivationFunctionType.Square,
                                     accum_out=A_all[:, j:j + 1])
            nc.vector.tensor_scalar_mul(negA[:], A_all[:], -1.0)

            # ---- binade select rows: sel[j, n] = (A >= 256) as fp16 rows
            psR = psSp.tile([NCHUNK, 128], F32, name="psR", tag="psT")
            nc.tensor.transpose(psR[:], A_all[:], id32[:])
            selr = scr.tile([NCHUNK, 128], F16, name="selr", tag="selr")
            nc.vector.tensor_scalar(out=selr[:], in0=psR[:], scalar1=256.0, scalar2=None,
                                    op0=mybir.AluOpType.is_ge)
            nc.sync.dma_start(dram["drsel"][:], selr[:])
            nc.sync.dma_start(
                sel2[1:2, :, :],
                dram["drsel"][:].rearrange("j n -> (j n)"))

            # ---- main loop: matmuls -> nd -> max -> max_index
            for j in range(NCHUNK):
                nsl = slice(128 * j, 128 * (j + 1))
                psM = psMp.tile([128, K], F32, name="psM")
                for kh in range(2):
                    ksl = slice(512 * kh, 512 * (kh + 1))
                    nc.tensor.matmul(psM[:, ksl], zh_lo[:, nsl], eh_lo[:, ksl], start=True, stop=False)
                    nc.tensor.matmul(psM[:, ksl], zh_hi[:, nsl], eh_hi[:, ksl], start=False, stop=False)
                    nc.tensor.matmul(psM[:, ksl], zh_lo[:, nsl], el_lo[:, ksl], start=False, stop=False)
                    nc.tensor.matmul(psM[:, ksl], zh_hi[:, nsl], el_hi[:, ksl], start=False, stop=False)
                    nc.tensor.matmul(psM[:, ksl], zl_lo[:, nsl], eh_lo[:, ksl], start=False, stop=False)
                    nc.tensor.matmul(psM[:, ksl], zl_hi[:, nsl], eh_hi[:, ksl], start=False, stop=False)
                    nc.tensor.matmul(psM[:, ksl], sel2[:, j, :], rb_t[:, ksl], start=False, stop=True)
                nd = ndp.tile([128, K], F32, name="nd", tag="nd")
                nc.scalar.activation(nd[:], psM[:], mybir.ActivationFunctionType.Identity,
                                     bias=negA[:, j:j + 1], scale=1.0)
                nc.vector.max(max8a[:, 8 * j:8 * (j + 1)], nd[:])
                nc.vector.max_index(idx8a[:, 8 * j:8 * (j + 1)], max8a[:, 8 * j:8 * (j + 1)], nd[:])

            # ---- loss partials: sum_j max (= -sum min d) per partition
            nc.vector.tensor_reduce(
                lossv[:], max8a[:].rearrange("p (j e) -> p j e", e=8)[:, :, 0],
                mybir.AxisListType.X, mybir.AluOpType.add)
            nc.sync.dma_start(dram["lossv"][:], lossv[:])

            # ---- indices out: u16 -> f32 -> transpose -> i32 -> DRAM
            nc.vector.tensor_copy(idxf[:], idx8a[:].rearrange("p (j e) -> p j e", e=8)[:, :, 0])
            psI = psSp.tile([NCHUNK, 128], F32, name="psI", tag="psT")
            nc.tensor.transpose(psI[:], idxf[:], id32[:])
            nc.vector.tensor_copy(idxT[:], psI[:])
            nc.sync.dma_start(dram["idx"][:], idxT[:])

            # ---- wrapped idx layout for ap_gather: wrp[p, c] = idx0[c*16 + p%16]
            idx0 = idx8a[:].rearrange("p (j e) -> p j e", e=8)[:, :, 0]
            for q in range(8):
                nc.sync.dma_start(
                    wrp16[:].rearrange("r (j q) -> r j q", j=NCHUNK)[:, :, q],
                    idx0[16 * q:16 * (q + 1), :])
            for g in range(8):
                nc.sync.dma_start(wrp128[16 * g:16 * (g + 1), :], wrp16[:])

            # ---- gather z_q^T (packed c-pairs), one call per batch
            nc.gpsimd.load_library(_apg_lib)
            for b in range(B_LOC):
                nc.gpsimd.ap_gather(
                    zqp[:, 1024 * b:1024 * (b + 1), :],
                    etab[:],
                    wrp128[:, 64 * b:64 * (b + 1)].bitcast(I16),
                    128, K, 2, 1024)
                for ci in range(2):
                    for hh in range(4):
                        cp0 = 32 * hh
                        nc.sync.dma_start(
                            dram["zq"][b][128 * ci + cp0:128 * ci + cp0 + 32, :],
                            zqp[cp0:cp0 + 32, 1024 * b:1024 * (b + 1), ci])

    nc.compile()
    return nc


_NC_CACHE = {}


def _get_nc():
    if "nc" not in _NC_CACHE:
        _NC_CACHE["nc"] = _build_nc()
    return _NC_CACHE["nc"]


def _host_prep(z, embedding):
    """Build per-core input maps."""
    z = np.ascontiguousarray(z, dtype=np.float32)        # [32, 256, 32, 32]
    e = np.ascontiguousarray(embedding, dtype=np.float32)  # [1024, 256]

    zf = z.reshape(B, D, H * W)
    zh = zf.astype(np.float16)
    zl = (zf - zh.astype(np.float32)).astype(np.float16)

    e2 = (2.0 * e).astype(np.float32)                     # exact
    e2T = np.ascontiguousarray(e2.T)                      # [256, 1024]
    ehT = e2T.astype(np.float16)
    elT = (e2T - ehT.astype(np.float32)).astype(np.float16)
    ehT2 = np.ascontiguousarray(ehT.reshape(2, 128, K))
    elT2 = np.ascontiguousarray(elT.reshape(2, 128, K))

    Bv = np.sum(e * e, axis=1, dtype=np.float32)          # [1024]
    rb1 = (np.round(Bv.astype(np.float64) / G1) * G1).astype(np.float32)
    rb2 = (np.round(Bv.astype(np.float64) / G2) * G2).astype(np.float32)
    rbrow = np.stack([-rb1, -(rb2 - rb1)]).astype(np.float16)  # exact in fp16

    eT = np.ascontiguousarray(e.T)  # [256, 1024]
    etab = np.ascontiguousarray(
        np.stack([eT[0:128], eT[128:256]], axis=-1)).astype(np.float32)  # [cp, k, ci] = e[k, cp + 128*ci]

    onesrow = np.ones((1, N_LOC), dtype=np.float16)
    ident16 = np.eye(128, dtype=np.float16)
    ident32 = np.eye(128, dtype=np.float32)

    in_maps = []
    for c in range(N_CORES):
        zsh = zh[B_LOC * c:B_LOC * (c + 1)]               # [4, 256, 1024]
        zsl = zl[B_LOC * c:B_LOC * (c + 1)]
        zh2 = np.ascontiguousarray(
            zsh.reshape(B_LOC, 2, 128, H * W).transpose(1, 2, 0, 3).reshape(2, 128, N_LOC))
        zl2 = np.ascontiguousarray(
            zsl.reshape(B_LOC, 2, 128, H * W).transpose(1, 2, 0, 3).reshape(2, 128, N_LOC))
        in_maps.append({
            "zh2": zh2, "zl2": zl2, "ehT": ehT2, "elT": elT2,
            "rbrow": rbrow, "onesrow": onesrow, "etab": etab,
            "ident16": ident16, "ident32": ident32,
        })
    return in_maps


def kernel(z, embedding):
    nc = _get_nc()
    in_maps = _host_prep(z, embedding)
    res = run_bass_kernel_spmd(nc, in_maps, core_ids=list(range(N_CORES)))
    results = res.results

    zq = np.concatenate([r["zq"] for r in results], axis=0).reshape(B, D, H, W)
    idx = np.concatenate([r["idx"].reshape(-1) for r in results]).astype(np.int32)
    sse = -sum(float(r["lossv"].sum(dtype=np.float64)) for r in results)
    loss = np.float32(1.25 * sse / (B * D * H * W))
    return zq, idx, loss


if __name__ == "__main__":
    rng = np.random.default_rng(0)
    z = rng.standard_normal((B, D, H, W)).astype(np.float32)
    e = rng.uniform(-1 / K, 1 / K, (K, D)).astype(np.float32)
    zq, idx, loss = kernel(z, e)
    print("shapes:", zq.shape, idx.shape, loss)
